# revision 42
# baseline (speedup 1.0000x reference)
"""Causal multi-head attention (16 heads, hd=64) on 8 trn2 NeuronCores.

Sharding: core c -> batch b = c // 4, head-group g = c % 4 (4 heads = 256
columns of Wq/Wk/Wv).  Each core computes its [S, 256] slice of the three
outputs (attn out, K_cache, V_cache); the host gathers slices.

Engine streams are STATIC (per-engine execution order == emission order),
so the kernel is emitted as one linear software pipeline clocked by the
exp chain on the scalar engine (the ~80us serial co-bottleneck):

  step (pair, qi, t):  [proj filler half-piece(s)]  scores(t)  exp(t)
                       [select]  [AV sweeps gated >=2 steps back]

  - All-bf16 dataflow: x/W in, out/kct/vc out (host casts); bf16 avoids
    the 4x fp32r penalty on narrow matmuls and gets FWL (2x LDWEIGHTS).
  - Inputs arrive in 6 large DMAs (rearranged dram APs) ordered by
    consumption -- per-piece dma_start issue cost (~0.64us) was
    rate-limiting delivery; consts ride the scalar queue.
  - A dummy activation preloads the exp table set (~2.7us) off the
    critical path.
  - K/Q projections: KT/QT [c, q], per-partition bias fused into the DVE
    eviction straight to bf16; ktb doubles as the K_cache DMA source.
  - V natural [s, c] (rank-1 bias matmul) -> vc DMA + per-head V_aug
    [k, 65] tiles with a ones column (denominator rides AV for free).
  - scores: two heads of a pair as concurrent row-tiled matmuls (K=64,
    partition offsets 0/64) into one [128, 1024] psum tile; diagonal
    blocks narrowed; exp on ACT -> bf16 pt; gpsimd affine_select zeroes
    the partial triangle (gpsimd carries nothing else).
  - AV sweep (pair, qc, h): out[q, d] += pt[t]^T @ V_aug[t] over t in
    one psum [128, 65] accumulation burst, emitted 2 exp-steps behind
    its gating exp so the burst never stalls mid-group; output lands in
    [q, d] layout (no PE transposes); reciprocal of the ones column +
    tensor_scalar_mul -> bf16 out staging -> DMA.
  - Proj filler is half-pieces (~4 matmuls, ~0.5-0.9us) on an explicit
    schedule: strictly piece-sequential (the proj psum pool has 2 bufs;
    >2 interleaved pieces deadlock the static PE queue), paced so each
    pipeline step carries roughly one exp-instruction's worth of PE
    work.
"""

import numpy as np

P = 128
S = 2048
HIN = 1024
C = 256  # columns per core = 4 heads * 64
HD = 64
NCORES = 8
HC = HIN // P  # 8 contraction chunks
NKT = S // P  # 16 k-tiles
QW = 512  # q-slice width
NQ = S // QW  # 4 q-slices
NPAIR = C // P  # 2 head-pairs per core

_nc_cache = None


def build_nc():
    import concourse.bacc as bacc
    import concourse.mybir as mybir
    from concourse.tile import TileContext
    from contextlib import ExitStack
    from collections import deque

    f32 = mybir.dt.float32
    bf16 = mybir.dt.bfloat16
    Exp = mybir.ActivationFunctionType.Exp
    is_ge = mybir.AluOpType.is_ge

    nc = bacc.Bacc(None, target_bir_lowering=False)

    xt = nc.declare_dram_parameter("xt", [HIN, S], bf16, isOutput=False)
    wq = nc.declare_dram_parameter("wq", [HIN, C], bf16, isOutput=False)
    wk = nc.declare_dram_parameter("wk", [HIN, C], bf16, isOutput=False)
    wv = nc.declare_dram_parameter("wv", [HIN, C], bf16, isOutput=False)
    bqc = nc.declare_dram_parameter("bqc", [P, NPAIR], f32, isOutput=False)
    bkc = nc.declare_dram_parameter("bkc", [P, NPAIR], f32, isOutput=False)
    bv = nc.declare_dram_parameter("bv", [1, C], bf16, isOutput=False)
    padneg = nc.declare_dram_parameter("padneg", [P, NKT], f32, isOutput=False)
    ones = nc.declare_dram_parameter("ones", [1, P], bf16, isOutput=False)
    out = nc.declare_dram_parameter("out", [S, C], bf16, isOutput=True)
    kct = nc.declare_dram_parameter("kct", [C, S], bf16, isOutput=True)
    vc = nc.declare_dram_parameter("vc", [S, C], bf16, isOutput=True)

    with TileContext(nc) as tc, ExitStack() as ctx:
        persist = ctx.enter_context(tc.tile_pool(name="persist", bufs=1))
        xt_sb = persist.tile([P, HC, S], bf16)
        wq_sb = persist.tile([P, HC, C], bf16)
        wk_sb = persist.tile([P, HC, C], bf16)
        wv_sb = persist.tile([P, HC, C], bf16)
        bqc_sb = persist.tile([P, NPAIR], f32)
        bkc_sb = persist.tile([P, NPAIR], f32)
        bv_sb = persist.tile([1, C], bf16)
        pn_sb = persist.tile([P, NKT], f32)
        ones_sb = persist.tile([1, P], bf16)
        qt_sb = persist.tile([P, NPAIR, S], bf16)
        ktb = persist.tile([P, NPAIR, S], bf16)
        va = persist.tile([P, NKT, 2 * NPAIR, HD + 1], bf16)
        out_sb = persist.tile([P, NKT, C], bf16)
        vsb = persist.tile([P, NKT, C], bf16)

        # inputs in 6 big DMAs on sync, ordered by consumption; consts
        # on the scalar queue (pad first -- it gates the exp-table
        # preload)
        q0 = slice(0, QW)
        nc.sync.dma_start(
            xt_sb[:, :, q0], xt[:, q0].rearrange("(j p) w -> p j w", p=P)
        )
        nc.sync.dma_start(wk_sb[:], wk[:].rearrange("(j p) c -> p j c", p=P))
        nc.sync.dma_start(wq_sb[:], wq[:].rearrange("(j p) c -> p j c", p=P))
        nc.sync.dma_start(wv_sb[:], wv[:].rearrange("(j p) c -> p j c", p=P))
        q1 = slice(QW, 2 * QW)
        nc.sync.dma_start(
            xt_sb[:, :, q1], xt[:, q1].rearrange("(j p) w -> p j w", p=P)
        )
        h2 = slice(S // 2, S)
        nc.sync.dma_start(
            xt_sb[:, :, h2], xt[:, h2].rearrange("(j p) w -> p j w", p=P)
        )
        nc.scalar.dma_start(pn_sb[:], padneg[:])
        nc.scalar.dma_start(bqc_sb[:], bqc[:])
        nc.scalar.dma_start(bkc_sb[:], bkc[:])
        nc.scalar.dma_start(bv_sb[:], bv[:])
        nc.scalar.dma_start(ones_sb[:], ones[:])
        # ones column of V_aug (position 64); DVE write, not DMA (a
        # strided sub-word DMA write would RMW-race adjacent columns)
        nc.vector.memset(va[:, :, :, HD : HD + 1], 1.0)

        psum = ctx.enter_context(tc.tile_pool(name="psum", bufs=2, space="PSUM"))
        work = ctx.enter_context(tc.tile_pool(name="work", bufs=2))

        # preload the exp table set (~2.7us) during the x-DMA lead-in so
        # the first real exp doesn't pay it on the critical path
        tblw = work.tile([1, 1], f32, tag="tblw", bufs=1, name="tblw")
        nc.scalar.activation(
            tblw[:], pn_sb[:1, 0:1], Exp, bias=pn_sb[:1, 0:1], scale=0.0
        )

        # clock-gate warm-up paced to END at input-DMA arrival (~15us):
        # 24 WAW-serialized N=128 dummies (~0.37us each) keep the PE
        # busy from ~6us so the HAM window lifts the clock to 2.4 GHz
        # before the first real projection -- ending early re-throttles
        # (the gate drops after ~3.4us idle), which is why earlier
        # warm-up attempts didn't help
        warm_sb = persist.tile([P, 2 * P], bf16)
        nc.vector.memset(warm_sb[:], 1.0)
        warm_ps = psum.tile([P, QW], f32, tag="proj", bufs=2, name="warm_ps")
        for _ in range(24):
            nc.tensor.matmul(
                warm_ps[:, :P], warm_sb[:, :P], warm_sb[:, P:],
                start=True, stop=True,
            )

        # ---- emission pieces (half-piece granular for filler pacing) --
        class KQPiece:
            def __init__(self, qi, p2, which):
                self.qi, self.p2, self.which = qi, p2, which
                self.j = 0
                self.ps = None

            def emit_half(self):
                qi, p2, which = self.qi, self.p2, self.which
                qsl = slice(qi * QW, (qi + 1) * QW)
                csl = slice(p2 * P, (p2 + 1) * P)
                w_sb, b_sb, dst = (
                    (wk_sb, bkc_sb, ktb)
                    if which == "k"
                    else (wq_sb, bqc_sb, qt_sb)
                )
                if self.ps is None:
                    self.ps = psum.tile(
                        [P, QW], f32, tag="proj", bufs=2, name="p_ps"
                    )
                for j in range(self.j, self.j + 4):
                    nc.tensor.matmul(
                        self.ps, w_sb[:, j, csl], xt_sb[:, j, qsl],
                        start=(j == 0), stop=(j == HC - 1),
                    )
                self.j += 4
                if self.j == HC:
                    nc.vector.tensor_scalar_add(
                        dst[:, p2, qsl], self.ps, b_sb[:, p2 : p2 + 1]
                    )
                    if which == "k":
                        nc.sync.dma_start(
                            kct[p2 * P : (p2 + 1) * P, qsl], ktb[:, p2, qsl]
                        )

        class VPiece:
            def __init__(self, i):
                self.i = i
                self.j = 0
                self.ps = None

            def emit_half(self):
                i = self.i
                ksl = slice(i * P, (i + 1) * P)
                if self.ps is None:
                    self.ps = psum.tile(
                        [P, QW], f32, tag="proj", bufs=2, name="v_ps"
                    )[:, :C]
                for j in range(self.j, min(self.j + 4, HC)):
                    nc.tensor.matmul(
                        self.ps, xt_sb[:, j, ksl], wv_sb[:, j, :],
                        start=(j == 0), stop=False,
                    )
                self.j += 4
                if self.j >= HC:
                    nc.tensor.matmul(
                        self.ps, ones_sb[:1, :P], bv_sb[:1, :],
                        start=False, stop=True,
                    )
                    nc.vector.tensor_copy(out=vsb[:, i, :], in_=self.ps)
                    nc.sync.dma_start(vc[ksl, :], vsb[:, i, :])
                    for h in range(2 * NPAIR):
                        nc.vector.tensor_copy(
                            out=va[:, i, h, 0:HD],
                            in_=vsb[:, i, h * HD : (h + 1) * HD],
                        )

        def scores_exp(qi, p2, t, pt):
            ksl = slice(t * P, (t + 1) * P)
            d = t - 4 * qi
            W = QW if d < 0 else QW - d * P
            off = 0 if d < 0 else d * P
            qg = qi * QW + off
            st = psum.tile([P, 2 * QW], f32, tag="st", bufs=2, name="st")
            nc.tensor.matmul(
                st[:, 0:W], ktb[0:HD, p2, ksl],
                qt_sb[0:HD, p2, qg : qg + W], start=True, stop=True,
            )
            nc.tensor.matmul(
                st[:, QW : QW + W], ktb[HD:P, p2, ksl],
                qt_sb[HD:P, p2, qg : qg + W], start=True, stop=True,
            )
            st3 = st[:].rearrange("p (h w) -> p h w", h=2)[:, :, 0:W]
            nc.scalar.activation(
                pt[:, t, :, off : off + W], st3, Exp,
                bias=pn_sb[:, t : t + 1], scale=0.125,
            )
            if d >= 0:
                nc.gpsimd.affine_select(
                    out=pt[:, t, :, off : off + P],
                    in_=pt[:, t, :, off : off + P],
                    compare_op=is_ge, fill=0.0, base=0,
                    pattern=[[0, 2], [1, P]], channel_multiplier=-1,
                )

        outcnt = {}

        def sweep(qi, p2, qc, pt):
            gq = 4 * qi + qc
            qoff = qc * P
            nt = gq + 1
            for h in range(2):
                av = psum.tile([P, QW], f32, tag="av", bufs=2, name="av")[
                    :, : HD + 1
                ]
                for t in range(nt):
                    nc.tensor.matmul(
                        av, pt[:, t, h, qoff : qoff + P],
                        va[:, t, 2 * p2 + h, :],
                        start=(t == 0), stop=(t == nt - 1),
                    )
                rcp = work.tile([P, 1], f32, tag="rcp", bufs=4, name="rcp")
                nc.vector.reciprocal(rcp[:], av[:, HD : HD + 1])
                col = (2 * p2 + h) * HD
                nc.vector.tensor_scalar_mul(
                    out_sb[:, gq, col : col + HD], av[:, 0:HD], rcp[:]
                )
            outcnt[gq] = outcnt.get(gq, 0) + 1
            if outcnt[gq] == NPAIR:
                nc.sync.dma_start(out[gq * P : (gq + 1) * P, :], out_sb[:, gq, :])

        # ---- the linear software pipeline ------------------------------
        # first pair's projections ahead of the pipeline; everything else
        # is paced as half-piece filler on an explicit schedule
        kq00k = KQPiece(0, 0, "k")
        kq00k.emit_half()
        kq00k.emit_half()
        kq00q = KQPiece(0, 0, "q")
        kq00q.emit_half()
        kq00q.emit_half()

        def halves(p):
            return [p, p]

        sched = {}

        def put(step, piece):
            sched.setdefault(step, []).append(piece)

        # fill phase: two halves per step (structurally dense -- proj(0)
        # remainder + proj(1) must precede their attention blocks)
        fill = (
            halves(KQPiece(0, 1, "k"))
            + halves(KQPiece(0, 1, "q"))
            + halves(VPiece(0))
            + halves(VPiece(1))
            + halves(VPiece(2))
            + halves(VPiece(3))
            + halves(KQPiece(1, 0, "k"))
            + halves(KQPiece(1, 0, "q"))
        )
        for k, piece in enumerate(fill):
            put(k // 2, piece)
        seq = [
            (8, KQPiece(1, 1, "k")),
            (9, KQPiece(1, 1, "k")),
            (10, KQPiece(1, 1, "q")),
            (11, KQPiece(1, 1, "q")),
            (12, VPiece(4)),
            (12, VPiece(4)),
            (13, VPiece(5)),
            (13, VPiece(5)),
            (14, VPiece(6)),
            (14, VPiece(6)),
            (15, VPiece(7)),
            (15, VPiece(7)),
            (16, KQPiece(2, 0, "k")),
            (17, KQPiece(2, 0, "k")),
            (18, KQPiece(2, 0, "q")),
            (19, KQPiece(2, 0, "q")),
            (24, KQPiece(2, 1, "k")),
            (25, KQPiece(2, 1, "k")),
            (26, KQPiece(2, 1, "q")),
            (27, KQPiece(2, 1, "q")),
            (28, VPiece(8)),
            (29, VPiece(8)),
            (30, VPiece(9)),
            (31, VPiece(9)),
            (32, VPiece(10)),
            (32, VPiece(10)),
            (33, VPiece(11)),
            (33, VPiece(11)),
            (40, KQPiece(3, 0, "k")),
            (41, KQPiece(3, 0, "k")),
            (42, KQPiece(3, 0, "q")),
            (43, KQPiece(3, 0, "q")),
            (52, KQPiece(3, 1, "k")),
            (53, KQPiece(3, 1, "k")),
            (54, KQPiece(3, 1, "q")),
            (55, KQPiece(3, 1, "q")),
            (57, VPiece(12)),
            (57, VPiece(12)),
            (58, VPiece(13)),
            (58, VPiece(13)),
            (59, VPiece(14)),
            (59, VPiece(14)),
            (60, VPiece(15)),
            (60, VPiece(15)),
        ]
        dedup = {}
        for s, piece in seq:
            key = (
                (type(piece).__name__,)
                + ((piece.qi, piece.p2, piece.which) if isinstance(piece, KQPiece) else (piece.i,))
            )
            if key in dedup:
                piece = dedup[key]
            else:
                dedup[key] = piece
            put(s, piece)

        pend = deque()  # (gate_step, qi, p2, qc, pt)
        step = 0
        LAG = 2
        for qi in range(NQ):
            for p2 in range(NPAIR):
                pt = work.tile([P, NKT, 2, QW], bf16, tag="pt", bufs=2, name="pt")
                for t in range(4 * qi + 4):
                    # scores FIRST: the exp chain is the serial
                    # bottleneck, and filler emitted ahead of the scores
                    # in the static PE stream delays the next exp by the
                    # filler's duration every step of the fill phase
                    scores_exp(qi, p2, t, pt)
                    for piece in sched.pop(step, ()):
                        piece.emit_half()
                    d = t - 4 * qi
                    if d >= 0:
                        pend.append((step, qi, p2, d, pt))
                    while pend and pend[0][0] <= step - LAG:
                        _, sqi, sp2, sqc, spt = pend.popleft()
                        sweep(sqi, sp2, sqc, spt)
                    step += 1
        while pend:
            _, sqi, sp2, sqc, spt = pend.popleft()
            sweep(sqi, sp2, sqc, spt)

    nc.finalize()
    return nc


def get_nc():
    global _nc_cache
    if _nc_cache is None:
        _nc_cache = build_nc()
    return _nc_cache


def make_in_maps(x, pad_mask, Wq, bq, Wk, bk, Wv, bv):
    import ml_dtypes

    bf = ml_dtypes.bfloat16
    x = np.asarray(x, np.float32)
    pad_mask = np.asarray(pad_mask, np.float32)
    Wq = np.asarray(Wq, np.float32)
    bq = np.asarray(bq, np.float32)
    Wk = np.asarray(Wk, np.float32)
    bk = np.asarray(bk, np.float32)
    Wv = np.asarray(Wv, np.float32)
    bv = np.asarray(bv, np.float32)
    in_maps = []
    for c in range(NCORES):
        b, g = divmod(c, 4)
        cols = slice(g * C, (g + 1) * C)
        xt = np.ascontiguousarray(x[b].T.astype(bf))  # [HIN, S]
        pn = ((pad_mask[b] - 1.0) * 1e6).reshape(NKT, P).T.copy()  # [P, NKT]
        in_maps.append(
            dict(
                xt=xt,
                ones=np.ones((1, P), bf),
                wq=np.ascontiguousarray(Wq[:, cols].astype(bf)),
                wk=np.ascontiguousarray(Wk[:, cols].astype(bf)),
                wv=np.ascontiguousarray(Wv[:, cols].astype(bf)),
                bqc=np.ascontiguousarray(bq[cols].reshape(NPAIR, P).T),
                bkc=np.ascontiguousarray(bk[cols].reshape(NPAIR, P).T),
                bv=np.ascontiguousarray(bv[cols].reshape(1, C).astype(bf)),
                padneg=pn,
            )
        )
    return in_maps


def gather(results):
    B = 2
    out = np.empty((B, S, HIN), np.float32)
    kcache = np.empty((B, S, HIN), np.float32)
    vcache = np.empty((B, S, HIN), np.float32)
    for c in range(NCORES):
        b, g = divmod(c, 4)
        cols = slice(g * C, (g + 1) * C)
        out[b, :, cols] = results[c]["out"].astype(np.float32)
        kcache[b, :, cols] = results[c]["kct"].T.astype(np.float32)
        vcache[b, :, cols] = results[c]["vc"].astype(np.float32)
    return out, kcache, vcache


def kernel(x, pad_mask, Wq, bq, Wk, bk, Wv, bv):
    from concourse.bass_utils import run_bass_kernel_spmd

    nc = get_nc()
    in_maps = make_in_maps(x, pad_mask, Wq, bq, Wk, bk, Wv, bv)
    res = run_bass_kernel_spmd(nc, in_maps, list(range(NCORES)))
    return gather(res.results)
